# revision 1
# baseline (speedup 1.0000x reference)
"""Cross-attention kernel for TRN2, SPMD over 8 NeuronCores.

Problem: B=8, SQ=4096, SKV=77, D_EMBED=1024, D_CROSS=768, H=16, DH=64.
  q = x @ wq + bq ; k = y @ wk + bk ; v = y @ wv + bv
  out = softmax(q k^T / 8) v @ wo + bo

Sharding: pure data-parallel over batch (1 batch element per core, no
collectives). Host pre-transposes x and y per core so the device kernel
keeps every tensor feature-major (contraction dim on partitions) until the
O-projection, which uses attnout^T as the stationary operand to emit the
output in natural row-major layout.

Compute dtype: bf16 operands (host-cast), fp32 PSUM accumulation, fp32 out.

Device layout per core (all DRAM tensors are per-core inputs):
  xt  (1024, 4096) bf16 : x[b]^T
  yt  (768, 77)    bf16 : y[b]^T
  wq/wo (1024,1024), wk/wv (768,1024) bf16
  bq/bv/bo (1, 1024) bf16 ; bk8 (1, 1024) f32 = bk * 0.125
  out (4096, 1024) f32

Softmax is computed without max-subtraction (scores are O(5) for this
problem class; exp stays comfortably inside fp32/bf16 range):
  scoresT[s,q] = k'_h @ q_h^T with k' = (k + bk)/8 folded at k-projection
  e = exp(scoresT)  (bf16)
  r = 1 / (ones77 @ e)          per-head [1, SQ] via PE column-sum
  aoT[d,q] = (v_h^T @ e) * broadcast(r)   (normalization commutes)
  out[q,:] = aoT^T @ wo + bo    (aoT tiles as stationary operand)
"""

import numpy as np
import ml_dtypes

import concourse.bass as bass
import concourse.mybir as mybir
import concourse.tile as tile
from concourse import bacc
from concourse import bass_utils

F32 = mybir.dt.float32
BF16 = mybir.dt.bfloat16
AF = mybir.ActivationFunctionType

B = 8
SQ = 4096
SKV = 77
D = 1024
DC = 768
H = 16
DH = 64
KT = D // 128    # 8 embed k-tiles
KC = DC // 128   # 6 cross k-tiles
CT = D // 128    # 8 column tiles of the 1024-wide projections
CH = 512         # query chunk
NCH = SQ // CH   # 8 chunks
NQT = CH // 128  # 4 query 128-tiles per chunk

_CACHED = {}


def _build():
    nc = bacc.Bacc("TRN2", target_bir_lowering=False, debug=False, num_devices=B)

    xt = nc.dram_tensor("xt", (D, SQ), BF16, kind="ExternalInput")
    yt = nc.dram_tensor("yt", (DC, SKV), BF16, kind="ExternalInput")
    wq_d = nc.dram_tensor("wq", (D, D), BF16, kind="ExternalInput")
    wk_d = nc.dram_tensor("wk", (DC, D), BF16, kind="ExternalInput")
    wv_d = nc.dram_tensor("wv", (DC, D), BF16, kind="ExternalInput")
    wo_d = nc.dram_tensor("wo", (D, D), BF16, kind="ExternalInput")
    bq_d = nc.dram_tensor("bq", (1, D), BF16, kind="ExternalInput")
    bk8_d = nc.dram_tensor("bk8", (1, D), F32, kind="ExternalInput")
    bv_d = nc.dram_tensor("bv", (1, D), BF16, kind="ExternalInput")
    bo_d = nc.dram_tensor("bo", (1, D), BF16, kind="ExternalInput")
    sel16_d = nc.dram_tensor("sel16", (SKV, H * 16), BF16, kind="ExternalInput")
    sel64_d = nc.dram_tensor("sel64", (16, H * DH), BF16, kind="ExternalInput")
    out_d = nc.dram_tensor("out", (SQ, D), F32, kind="ExternalOutput")

    with tile.TileContext(nc) as tc:
        with (
            tc.tile_pool(name="consts", bufs=1) as consts,
            tc.tile_pool(name="wpool", bufs=1) as wpool,
            tc.tile_pool(name="xpool", bufs=2) as xpool,
            tc.tile_pool(name="qpool", bufs=2) as qpool,
            tc.tile_pool(name="epool", bufs=2) as epool,
            tc.tile_pool(name="rp", bufs=4) as rp,
            tc.tile_pool(name="rbpool", bufs=2) as rbpool,
            tc.tile_pool(name="aopool", bufs=2) as aopool,
            tc.tile_pool(name="opool", bufs=3) as opool,
            tc.tile_pool(name="pmm", bufs=2, space="PSUM") as pmm,
            tc.tile_pool(name="psc", bufs=2, space="PSUM") as psc,
            tc.tile_pool(name="ppv", bufs=2, space="PSUM") as ppv,
            tc.tile_pool(name="pnrm", bufs=2, space="PSUM") as pnrm,
        ):
            # ---- constants / weights ----
            wq_sb = wpool.tile([128, KT, D], BF16, tag="wq")
            nc.sync.dma_start(wq_sb[:], wq_d.ap().rearrange("(kt p) n -> p kt n", p=128))
            wo_sb = wpool.tile([128, KT, D], BF16, tag="wo")
            nc.sync.dma_start(wo_sb[:], wo_d.ap().rearrange("(kt p) n -> p kt n", p=128))
            wk_sb = wpool.tile([128, KC, D], BF16, tag="wk")
            nc.sync.dma_start(wk_sb[:], wk_d.ap().rearrange("(kt p) n -> p kt n", p=128))
            wv_sb = wpool.tile([128, KC, D], BF16, tag="wv")
            nc.sync.dma_start(wv_sb[:], wv_d.ap().rearrange("(kt p) n -> p kt n", p=128))

            yt_sb = consts.tile([128, KC, SKV], BF16, tag="yt")
            nc.sync.dma_start(yt_sb[:], yt.ap().rearrange("(kt p) s -> p kt s", p=128))

            bq_sb = consts.tile([1, D], BF16, tag="bq")
            nc.sync.dma_start(bq_sb[:], bq_d.ap())
            bv_sb = consts.tile([1, D], BF16, tag="bv")
            nc.sync.dma_start(bv_sb[:], bv_d.ap())
            bo_sb = consts.tile([1, D], BF16, tag="bo")
            nc.sync.dma_start(bo_sb[:], bo_d.ap())
            bk8_sb = consts.tile([128, CT], F32, tag="bk8")
            nc.sync.dma_start(bk8_sb[:], bk8_d.ap().rearrange("a (ct p) -> (a p) ct", p=128))

            ones_q = consts.tile([1, CH], BF16, tag="ones_q")
            nc.vector.memset(ones_q[:], 1.0)
            ones77r = consts.tile([1, SKV], BF16, tag="ones77r")
            nc.vector.memset(ones77r[:], 1.0)
            ones1 = consts.tile([1, 128], BF16, tag="ones1")
            nc.vector.memset(ones1[:], 1.0)
            sel16_sb = consts.tile([SKV, H * 16], BF16, tag="sel16")
            nc.sync.dma_start(sel16_sb[:], sel16_d.ap())
            sel64_sb = consts.tile([16, H * DH], BF16, tag="sel64")
            nc.sync.dma_start(sel64_sb[:], sel64_d.ap())

            kT_sb = consts.tile([128, CT, SKV], BF16, tag="kT")
            # filled after v projection: ones column for fused exp-sums
            v_aug = consts.tile([SKV, H, DH + 1], BF16, tag="v")

            # ---- k projection: kT[c, s] = sum_k wk[k, c] yT[k, s]; fold (.+bk)/8 ----
            for ct in range(CT):
                psk = pmm.tile([128, CH], F32, tag="mm")
                for kt in range(KC):
                    nc.tensor.matmul(
                        psk[:, 0:SKV],
                        wk_sb[:, kt, ct * 128:(ct + 1) * 128],
                        yt_sb[:, kt, :],
                        start=(kt == 0),
                        stop=(kt == KC - 1),
                    )
                nc.scalar.activation(
                    kT_sb[:, ct, :],
                    psk[:, 0:SKV],
                    AF.Identity,
                    scale=0.125,
                    bias=bk8_sb[:, ct:ct + 1],
                )

            # ---- v projection: v[s, c] = sum_k yT[k, s] wv[k, c] + bv[c] ----
            for n in range(2):
                psv = pmm.tile([128, CH], F32, tag="mm")
                for kt in range(KC):
                    nc.tensor.matmul(
                        psv[0:SKV, :],
                        yt_sb[:, kt, :],
                        wv_sb[:, kt, n * 512:(n + 1) * 512],
                        start=(kt == 0),
                        stop=False,
                    )
                nc.tensor.matmul(
                    psv[0:SKV, :],
                    ones77r[:],
                    bv_sb[0:1, n * 512:(n + 1) * 512],
                    start=False,
                    stop=True,
                )
                for j in range(8):
                    h = n * 8 + j
                    nc.any.tensor_copy(v_aug[:, h, 0:DH], psv[0:SKV, j * DH:(j + 1) * DH])

            nc.vector.memset(v_aug[:, :, DH:DH + 1], 1.0)

            # ---- main loop over query chunks ----
            for c in range(NCH):
                q0 = c * CH

                xT_ch = xpool.tile([128, KT, CH], BF16, tag="xT")
                nc.sync.dma_start(
                    xT_ch[:],
                    xt.ap().rearrange("(kt p) q -> p kt q", p=128)[:, :, q0:q0 + CH],
                )

                # q^T projection (per column-tile), bias via rank-1 matmul
                qT = qpool.tile([128, CT, CH], BF16, tag="qT")
                for ct in range(CT):
                    psq = pmm.tile([128, CH], F32, tag="mm")
                    for kt in range(KT):
                        nc.tensor.matmul(
                            psq[:],
                            wq_sb[:, kt, ct * 128:(ct + 1) * 128],
                            xT_ch[:, kt, :],
                            start=(kt == 0),
                            stop=False,
                        )
                    nc.tensor.matmul(
                        psq[:],
                        bq_sb[0:1, ct * 128:(ct + 1) * 128],
                        ones_q[:],
                        start=False,
                        stop=True,
                    )
                    nc.vector.tensor_copy(qT[:, ct, :], psq[:])

                # attention pass A: scores -> exp -> sum-collect [16, CH]
                e_ch = epool.tile([SKV, H, CH], BF16, tag="e")
                ps_sum = pnrm.tile([16, CH], F32, tag="nrm")
                for h in range(H):
                    pssc = psc.tile([SKV, CH], F32, tag="sc")
                    nc.tensor.matmul(
                        pssc[:],
                        kT_sb[(h % 2) * 64:(h % 2) * 64 + 64, h // 2, :],
                        qT[(h % 2) * 64:(h % 2) * 64 + 64, h // 2, :],
                        start=True, stop=True,
                    )
                    nc.scalar.activation(e_ch[:, h, :], pssc[:], AF.Exp)
                    nc.tensor.matmul(
                        ps_sum[:], sel16_sb[:, h * 16:(h + 1) * 16], e_ch[:, h, :],
                        start=(h == 0), stop=(h == H - 1), skip_group_check=True,
                    )
                r16 = rp.tile([16, CH], BF16, tag="r")
                with nc.allow_low_precision(reason="softmax recip in bf16"):
                    nc.vector.reciprocal(r16[:], ps_sum[:])

                # pass B: broadcast 1/sum, PV, normalize
                aoT = aopool.tile([128, KT, CH], BF16, tag="aoT")
                for hp in range(H // 2):
                    rb_ps = pnrm.tile([128, CH], F32, tag="nrm")
                    for half in range(2):
                        h = 2 * hp + half
                        nc.tensor.matmul(
                            rb_ps[half * 64:(half + 1) * 64, :],
                            sel64_sb[:, h * 64:(h + 1) * 64],
                            r16[:],
                            start=True, stop=True,
                        )
                    rb_sb = rbpool.tile([128, CH], F32, tag="rb")
                    nc.scalar.activation(rb_sb[:], rb_ps[:], AF.Identity)
                    pspv = ppv.tile([128, CH], F32, tag="pv")
                    for half in range(2):
                        h = 2 * hp + half
                        nc.tensor.matmul(
                            pspv[half * 64:(half + 1) * 64, :],
                            v_aug[:, h, 0:DH],
                            e_ch[:, h, :],
                            start=True, stop=True,
                        )
                    for half in range(2):
                        nc.vector.tensor_mul(
                            aoT[half * 64:(half + 1) * 64, hp, :],
                            pspv[half * 64:(half + 1) * 64, :],
                            rb_sb[half * 64:(half + 1) * 64, :],
                        )

                # output projection: out[q, n] = sum_kt aoT[kt, q]^T wo[kt, n] + bo
                for qt in range(NQT):
                    for n in range(2):
                        pso = pmm.tile([128, 512], F32, tag="mm")
                        for kt in range(KT):
                            nc.tensor.matmul(
                                pso[:],
                                aoT[:, kt, qt * 128:(qt + 1) * 128],
                                wo_sb[:, kt, n * 512:(n + 1) * 512],
                                start=(kt == 0),
                                stop=False,
                            )
                        nc.tensor.matmul(
                            pso[:],
                            ones1[:],
                            bo_sb[0:1, n * 512:(n + 1) * 512],
                            start=False,
                            stop=True,
                        )
                        o_sb = opool.tile([128, 512], F32, tag="o")
                        nc.vector.tensor_copy(o_sb[:], pso[:])
                        nc.sync.dma_start(
                            out_d.ap()[q0 + qt * 128: q0 + (qt + 1) * 128,
                                       n * 512:(n + 1) * 512],
                            o_sb[:],
                        )

    nc.compile()
    return nc


def _get_nc():
    if "nc" not in _CACHED:
        _CACHED["nc"] = _build()
    return _CACHED["nc"]


def kernel(x, y, wq, bq, wk, bk, wv, bv, wo, bo):
    x = np.asarray(x)
    y = np.asarray(y)
    bf = ml_dtypes.bfloat16
    wq_b = np.asarray(wq).astype(bf)
    wk_b = np.asarray(wk).astype(bf)
    wv_b = np.asarray(wv).astype(bf)
    wo_b = np.asarray(wo).astype(bf)
    bq_b = np.asarray(bq).reshape(1, D).astype(bf)
    bv_b = np.asarray(bv).reshape(1, D).astype(bf)
    bo_b = np.asarray(bo).reshape(1, D).astype(bf)
    bk8 = (np.asarray(bk).reshape(1, D) * 0.125).astype(np.float32)
    sel16 = np.zeros((SKV, H, 16), np.float32)
    sel16[:, np.arange(H), np.arange(16)] = 1.0
    sel16 = sel16.reshape(SKV, H * 16).astype(bf)
    sel64 = np.zeros((16, H, DH), np.float32)
    sel64[np.arange(16), np.arange(H), :] = 1.0
    sel64 = sel64.reshape(16, H * DH).astype(bf)

    in_maps = []
    for b in range(B):
        in_maps.append({
            "xt": np.ascontiguousarray(x[b].T).astype(bf),
            "yt": np.ascontiguousarray(y[b].T).astype(bf),
            "wq": wq_b, "wk": wk_b, "wv": wv_b, "wo": wo_b,
            "bq": bq_b, "bk8": bk8, "bv": bv_b, "bo": bo_b,
            "sel16": sel16, "sel64": sel64,
        })

    nc = _get_nc()
    res = bass_utils.run_bass_kernel_spmd(nc, in_maps, core_ids=list(range(B)))
    out = np.stack([res.results[b]["out"] for b in range(B)], axis=0)
    return out.astype(np.float32)



# revision 16
# speedup vs baseline: 1.4729x; 1.4729x over previous
"""Cross-attention kernel for TRN2, SPMD over 8 NeuronCores.

Problem: B=8, SQ=4096, SKV=77, D_EMBED=1024, D_CROSS=768, H=16, DH=64.
  q = x @ wq + bq ; k = y @ wk + bk ; v = y @ wv + bv
  out = softmax(q k^T / 8) v @ wo + bo

Sharding: pure data-parallel over batch (1 batch element per core, no
collectives). Host pre-transposes x and y per core so the device kernel
keeps every tensor feature-major (contraction dim on partitions) until the
O-projection, which uses attnout^T as the stationary operand to emit the
output in natural row-major layout.

Compute dtype: bf16 operands (host-cast), fp32 PSUM accumulation, fp32 out.

PE-minimizing restructure vs the straightforward version:
  - Q bias folded into the PSUM->SBUF activation copy (per-partition bias);
    no rank-1 bias matmuls.
  - O bias folded into the V bias on the host: since softmax rows sum to 1
    per head, adding delta = bo @ wo^-1 to every v row adds bo to the output.
  - softmax normalization: 1/rowsum broadcast across partitions is done by
    one SBUF->SBUF replicating DMA per chunk (stride-0 source dims) instead
    of PE sel-matrix matmuls.
  - software pipelining across the 8 query chunks keeps the PE continuously
    busy (p-state stays at max clock):
      phase A(c): scores(c)+expsums(c) interleaved with Oproj(c-1)
      phase B(c): PV(c) interleaved with Qproj(c+1)

Softmax is computed without max-subtraction (scores are O(5) for this
problem class; exp stays comfortably inside fp32/bf16 range):
  scoresT[s,q] = k'_h @ q_h^T with k' = (k + bk)/8 folded at k-projection
  e = exp(scoresT)  (bf16)
  r = 1 / (sel16 @ e)           per-head rowsums collected to [16, SQch]
  aoT[d,q] = (v_h^T @ e) * bcast(r)
  out[q,:] = aoT^T @ wo         (aoT tiles as stationary operand)
"""

import numpy as np
import ml_dtypes

import concourse.bass as bass
import concourse.mybir as mybir
import concourse.tile as tile
from concourse import bacc
from concourse import bass_utils

F32 = mybir.dt.float32
BF16 = mybir.dt.bfloat16
AF = mybir.ActivationFunctionType

B = 8
SQ = 4096
SKV = 77
D = 1024
DC = 768
H = 16
DH = 64
KT = D // 128    # 8 embed k-tiles
KC = DC // 128   # 6 cross k-tiles
CT = D // 128    # 8 column tiles of the 1024-wide projections
CH = 512         # query chunk
NCH = SQ // CH   # 8 chunks
NQT = CH // 128  # 4 query 128-tiles per chunk

_CACHED = {}


def _build():
    nc = bacc.Bacc("TRN2", target_bir_lowering=False, debug=False, num_devices=B)

    xt = nc.dram_tensor("xt", (D, SQ), BF16, kind="ExternalInput")
    yt = nc.dram_tensor("yt", (DC, SKV), BF16, kind="ExternalInput")
    wq_d = nc.dram_tensor("wq", (D, D), BF16, kind="ExternalInput")
    wk_d = nc.dram_tensor("wk", (DC, D), BF16, kind="ExternalInput")
    wv_d = nc.dram_tensor("wv", (DC, D), BF16, kind="ExternalInput")
    wo_d = nc.dram_tensor("wo", (D, D), BF16, kind="ExternalInput")
    bq_d = nc.dram_tensor("bq", (1, D), F32, kind="ExternalInput")
    bk8_d = nc.dram_tensor("bk8", (1, D), F32, kind="ExternalInput")
    bvd_d = nc.dram_tensor("bvd", (1, D), BF16, kind="ExternalInput")
    sel16_d = nc.dram_tensor("sel16", (SKV, H * 16), BF16, kind="ExternalInput")
    rs_d = [nc.dram_tensor(f"rscratch{i}", (16, CH), BF16, kind="Internal")
            for i in range(2)]
    out_d = nc.dram_tensor("out", (SQ, D), F32, kind="ExternalOutput")

    with tile.TileContext(nc) as tc:
        with (
            tc.tile_pool(name="consts", bufs=1) as consts,
            tc.tile_pool(name="wpool", bufs=1) as wpool,
            tc.tile_pool(name="xpool", bufs=2) as xpool,
            tc.tile_pool(name="qpool", bufs=2) as qpool,
            tc.tile_pool(name="epool", bufs=1) as epool,
            tc.tile_pool(name="rp", bufs=2) as rp,
            tc.tile_pool(name="rbpool", bufs=1) as rbpool,
            tc.tile_pool(name="aopool", bufs=2) as aopool,
            tc.tile_pool(name="opool", bufs=3) as opool,
            tc.tile_pool(name="pmm", bufs=2, space="PSUM") as pmm,
            tc.tile_pool(name="psc", bufs=3, space="PSUM") as psc,
            tc.tile_pool(name="ppv", bufs=2, space="PSUM") as ppv,
            tc.tile_pool(name="pnrm", bufs=1, space="PSUM") as pnrm,
        ):
            # ---- constants / weights (wq + first x chunk first: Qproj(0)
            # is the first PE work after the tiny k/v projections) ----
            wq_sb = wpool.tile([128, KT, D], BF16, tag="wq")
            nc.sync.dma_start(wq_sb[:], wq_d.ap().rearrange("(kt p) n -> p kt n", p=128))

            xT0 = xpool.tile([128, KT, CH], BF16, tag="xT")
            nc.sync.dma_start(
                xT0[:], xt.ap().rearrange("(kt p) q -> p kt q", p=128)[:, :, 0:CH]
            )

            yt_sb = consts.tile([128, KC, SKV], BF16, tag="yt")
            nc.sync.dma_start(yt_sb[:], yt.ap().rearrange("(kt p) s -> p kt s", p=128))
            wk_sb = wpool.tile([128, KC, D], BF16, tag="wk")
            nc.sync.dma_start(wk_sb[:], wk_d.ap().rearrange("(kt p) n -> p kt n", p=128))
            wv_sb = wpool.tile([128, KC, D], BF16, tag="wv")
            nc.sync.dma_start(wv_sb[:], wv_d.ap().rearrange("(kt p) n -> p kt n", p=128))
            wo_sb = wpool.tile([128, KT, D], BF16, tag="wo")
            nc.sync.dma_start(wo_sb[:], wo_d.ap().rearrange("(kt p) n -> p kt n", p=128))

            bq_sb = consts.tile([128, CT], F32, tag="bq")
            nc.sync.dma_start(bq_sb[:], bq_d.ap().rearrange("a (ct p) -> (a p) ct", p=128))
            bk8_sb = consts.tile([128, CT], F32, tag="bk8")
            nc.sync.dma_start(bk8_sb[:], bk8_d.ap().rearrange("a (ct p) -> (a p) ct", p=128))
            bvd_sb = consts.tile([1, D], BF16, tag="bvd")
            nc.sync.dma_start(bvd_sb[:], bvd_d.ap())
            sel16_sb = consts.tile([SKV, H * 16], BF16, tag="sel16")
            nc.sync.dma_start(sel16_sb[:], sel16_d.ap())

            ones77r = consts.tile([1, SKV], BF16, tag="ones77r")
            nc.vector.memset(ones77r[:], 1.0)

            kT_sb = consts.tile([128, CT, SKV], BF16, tag="kT")
            v_sb = consts.tile([SKV, H, DH], BF16, tag="v")

            # ---- k projection: kT[c, s] = sum_k wk[k, c] yT[k, s]; fold (.+bk)/8 ----
            for ct in range(CT):
                psk = pmm.tile([128, CH], F32, tag="mm")
                for kt in range(KC):
                    nc.tensor.matmul(
                        psk[:, 0:SKV],
                        wk_sb[:, kt, ct * 128:(ct + 1) * 128],
                        yt_sb[:, kt, :],
                        start=(kt == 0),
                        stop=(kt == KC - 1),
                    )
                nc.scalar.activation(
                    kT_sb[:, ct, :],
                    psk[:, 0:SKV],
                    AF.Identity,
                    scale=0.125,
                    bias=bk8_sb[:, ct:ct + 1],
                )

            # ---- v projection: v[s, c] = sum_k yT[k, s] wv[k, c] + bvd[c] ----
            for n in range(2):
                psv = pmm.tile([128, CH], F32, tag="mm")
                for kt in range(KC):
                    nc.tensor.matmul(
                        psv[0:SKV, :],
                        yt_sb[:, kt, :],
                        wv_sb[:, kt, n * 512:(n + 1) * 512],
                        start=(kt == 0),
                        stop=False,
                    )
                nc.tensor.matmul(
                    psv[0:SKV, :],
                    ones77r[:],
                    bvd_sb[0:1, n * 512:(n + 1) * 512],
                    start=False,
                    stop=True,
                )
                for j in range(8):
                    h = n * 8 + j
                    nc.any.tensor_copy(v_sb[:, h, 0:DH], psv[0:SKV, j * DH:(j + 1) * DH])

            # ---- Qproj emitter: 8 ct-groups of (8 matmuls + activation) ----
            def qproj_group(ct, xT_ch, qT):
                psq = pmm.tile([128, CH], F32, tag="mm")
                for kt in range(KT):
                    nc.tensor.matmul(
                        psq[:],
                        wq_sb[:, kt, ct * 128:(ct + 1) * 128],
                        xT_ch[:, kt, :],
                        start=(kt == 0),
                        stop=(kt == KT - 1),
                    )
                nc.scalar.activation(
                    qT[:, ct, :], psq[:], AF.Identity, bias=bq_sb[:, ct:ct + 1]
                )

            # ---- Oproj emitter: generator yielding after every matmul ----
            def oproj_steps(cc, aoT):
                q0 = cc * CH
                for qt in range(NQT):
                    for n in range(2):
                        pso = pmm.tile([128, 512], F32, tag="mm")
                        for kt in range(KT):
                            nc.tensor.matmul(
                                pso[:],
                                aoT[:, kt, qt * 128:(qt + 1) * 128],
                                wo_sb[:, kt, n * 512:(n + 1) * 512],
                                start=(kt == 0),
                                stop=(kt == KT - 1),
                            )
                            yield
                        o_sb = opool.tile([128, 512], F32, tag="o")
                        nc.vector.tensor_copy(o_sb[:], pso[:])
                        nc.sync.dma_start(
                            out_d.ap()[q0 + qt * 128: q0 + (qt + 1) * 128,
                                       n * 512:(n + 1) * 512],
                            o_sb[:],
                        )

            def drain(it):
                if it is not None:
                    for _ in it:
                        pass

            # ---- prologue: Qproj(0) ----
            qT_cur = qpool.tile([128, CT, CH], BF16, tag="qT")
            for ct in range(CT):
                qproj_group(ct, xT0, qT_cur)
            xT_cur = xT0

            ao_prev = None  # aoT of chunk c-1
            o_iter = None   # in-flight Oproj step generator for chunk c-1

            for c in range(NCH):
                # ---------- phase A(c): scores+sums(c) ⋈ Oproj(c-1) ----------
                if c + 1 < NCH:
                    xT_next = xpool.tile([128, KT, CH], BF16, tag="xT")
                    nc.sync.dma_start(
                        xT_next[:],
                        xt.ap().rearrange("(kt p) q -> p kt q", p=128)
                        [:, :, (c + 1) * CH:(c + 2) * CH],
                    )
                else:
                    xT_next = None

                e_ch = epool.tile([SKV, H, CH], BF16, tag="e")
                ps_sum = pnrm.tile([16, CH], F32, tag="nrm")
                o_iter = oproj_steps(c - 1, ao_prev) if ao_prev is not None else None

                for h in range(H):
                    pssc = psc.tile([SKV, CH], F32, tag="sc")
                    nc.tensor.matmul(
                        pssc[:],
                        kT_sb[(h % 2) * 64:(h % 2) * 64 + 64, h // 2, :],
                        qT_cur[(h % 2) * 64:(h % 2) * 64 + 64, h // 2, :],
                        start=True, stop=True, skip_group_check=True,
                    )
                    nc.scalar.activation(e_ch[:, h, :], pssc[:], AF.Exp)
                    if h > 0:
                        nc.tensor.matmul(
                            ps_sum[:],
                            sel16_sb[:, (h - 1) * 16:h * 16],
                            e_ch[:, h - 1, :],
                            start=(h == 1), stop=False, skip_group_check=True,
                        )
                    if o_iter is not None:
                        for _ in range(4):
                            next(o_iter, None)
                nc.tensor.matmul(
                    ps_sum[:],
                    sel16_sb[:, (H - 1) * 16:H * 16],
                    e_ch[:, H - 1, :],
                    start=False, stop=True, skip_group_check=True,
                )
                drain(o_iter)
                o_iter = None

                r16 = rp.tile([16, CH], BF16, tag="r")
                with nc.allow_low_precision(reason="softmax recip in bf16"):
                    nc.vector.reciprocal(r16[:], ps_sum[:])

                # broadcast r16 rows across partitions via a DRAM round
                # trip: SBUF->DRAM, then two replicating DRAM->SBUF DMAs
                # (stride-0 partition source; legal for DRAM APs and honored
                # by hardware, unlike gpsimd partition_broadcast offsets).
                # r16 row layout (set via sel16 on the host): rows 0..7 =
                # even heads hp, rows 8..15 = odd heads hp.
                rs = rs_d[c % 2]
                nc.sync.dma_start(rs.ap(), r16[:])
                rb_all = rbpool.tile([128, H // 2, CH], BF16, tag="rb")
                nc.sync.dma_start(
                    rb_all[0:64, :, :],
                    rs.ap()[0:8, :].unsqueeze(0).broadcast_to([64, H // 2, CH]),
                )
                nc.sync.dma_start(
                    rb_all[64:128, :, :],
                    rs.ap()[8:16, :].unsqueeze(0).broadcast_to([64, H // 2, CH]),
                )

                # ---------- phase B(c): PV(c) ⋈ Qproj(c+1) ----------
                if xT_next is not None:
                    qT_next = qpool.tile([128, CT, CH], BF16, tag="qT")
                else:
                    qT_next = None

                aoT = aopool.tile([128, KT, CH], BF16, tag="aoT")
                for hp in range(H // 2):
                    pspv = ppv.tile([128, CH], F32, tag="pv")
                    for half in range(2):
                        h = 2 * hp + half
                        nc.tensor.matmul(
                            pspv[half * 64:(half + 1) * 64, :],
                            v_sb[:, h, 0:DH],
                            e_ch[:, h, :],
                            start=True, stop=True, skip_group_check=True,
                        )
                    # two half-muls so the even-head half doesn't wait for the
                    # odd-head broadcast
                    nc.vector.tensor_mul(
                        aoT[0:64, hp, :], pspv[0:64, :], rb_all[0:64, hp, :]
                    )
                    nc.vector.tensor_mul(
                        aoT[64:128, hp, :], pspv[64:128, :], rb_all[64:128, hp, :]
                    )
                    if qT_next is not None:
                        qproj_group(hp, xT_next, qT_next)

                ao_prev = aoT
                qT_cur = qT_next
                xT_cur = xT_next

            # ---------- epilogue: Oproj(7) ----------
            drain(oproj_steps(NCH - 1, ao_prev))

    nc.compile()
    return nc


def _get_nc():
    if "nc" not in _CACHED:
        _CACHED["nc"] = _build()
    return _CACHED["nc"]


def _build_in_maps(x, y, wq, bq, wk, bk, wv, bv, wo, bo):
    x = np.asarray(x)
    y = np.asarray(y)
    bf = ml_dtypes.bfloat16
    wq_b = np.asarray(wq).astype(bf)
    wk_b = np.asarray(wk).astype(bf)
    wv_b = np.asarray(wv).astype(bf)
    wo_b = np.asarray(wo).astype(bf)
    bq_f = np.asarray(bq).reshape(1, D).astype(np.float32)
    bk8 = (np.asarray(bk).reshape(1, D) * 0.125).astype(np.float32)
    # fold bo into the v bias: softmax rows sum to 1 per head, so adding
    # delta = bo @ wo^-1 to every v row adds (1 @ delta) @ wo = bo to out.
    bo64 = np.asarray(bo).astype(np.float64)
    if np.any(bo64 != 0.0):
        delta = np.linalg.solve(np.asarray(wo).astype(np.float64).T, bo64)
    else:
        delta = np.zeros((D,), np.float64)
    bvd = (np.asarray(bv).astype(np.float64) + delta).astype(np.float32)
    bvd_b = bvd.reshape(1, D).astype(bf)
    # head h's exp-sum is collected into r16 row (h%2)*8 + h//2 so that the
    # rb broadcast DMAs can read even/odd head blocks contiguously.
    rowmap = (np.arange(H) % 2) * 8 + np.arange(H) // 2
    sel16 = np.zeros((SKV, H, 16), np.float32)
    sel16[:, np.arange(H), rowmap] = 1.0
    sel16 = sel16.reshape(SKV, H * 16).astype(bf)

    in_maps = []
    for b in range(B):
        in_maps.append({
            "xt": np.ascontiguousarray(x[b].T).astype(bf),
            "yt": np.ascontiguousarray(y[b].T).astype(bf),
            "wq": wq_b, "wk": wk_b, "wv": wv_b, "wo": wo_b,
            "bq": bq_f, "bk8": bk8, "bvd": bvd_b,
            "sel16": sel16,
        })
    return in_maps


def kernel(x, y, wq, bq, wk, bk, wv, bv, wo, bo):
    in_maps = _build_in_maps(x, y, wq, bq, wk, bk, wv, bv, wo, bo)
    nc = _get_nc()
    res = bass_utils.run_bass_kernel_spmd(nc, in_maps, core_ids=list(range(B)))
    out = np.stack([res.results[b]["out"] for b in range(B)], axis=0)
    return out.astype(np.float32)
